# revision 1
# baseline (speedup 1.0000x reference)
"""ConvVQVAE forward on 8 Trainium2 NeuronCores — pure data parallel.

Shards batch B=64 as 8 images/core, replicates all params, runs the full
forward on each core, gathers outputs and finishes the (scalar) loss
reduction on host. Self-contained: hardcodes shapes from the problem spec.
"""
import numpy as np
import jax
import jax.numpy as jnp

# ---- fixed problem shapes ----
B, CH, H, W = 64, 3, 128, 128
K, D = 512, 64
NCORES = 8
EPS = 1e-5
DN = ('NCHW', 'OIHW', 'NCHW')


def _conv(x, w, stride=1, pad=0):
    return jax.lax.conv_general_dilated(
        x, w, (stride, stride), [(pad, pad), (pad, pad)], dimension_numbers=DN)


def _deconv(x, w, pad_lo, pad_hi):
    w_t = jnp.flip(w, (2, 3)).transpose(1, 0, 2, 3)
    return jax.lax.conv_general_dilated(
        x, w_t, (1, 1), [(pad_lo, pad_hi), (pad_lo, pad_hi)],
        lhs_dilation=(2, 2), dimension_numbers=DN)


def _bn(x, g, b):
    scale = (g / jnp.sqrt(1.0 + EPS))[None, :, None, None]
    return x * scale + b[None, :, None, None]


def _gelu(x):
    return jax.nn.gelu(x, approximate=False)


def _block(x, p):
    out = jax.nn.relu(_bn(_conv(x, p['conv1'], 1, 1), p['bn1_g'], p['bn1_b']))
    out = _bn(_conv(out, p['conv2'], 1, 1), p['bn2_g'], p['bn2_b'])
    if 'sc_conv' in p:
        sc = _bn(_conv(x, p['sc_conv'], 1, 0), p['sc_g'], p['sc_b'])
    else:
        sc = x
    return jax.nn.relu(out + sc)


def _shard_forward(x, params):
    """Forward for one batch shard; returns partial sums for the loss."""
    h = _gelu(_conv(x, params['ec1'], 2, 1))
    h = _gelu(_conv(h, params['ec2'], 2, 1))
    h = _gelu(_block(h, params['be1']))
    h = _gelu(_block(h, params['be2']))
    z_e = _conv(h, params['ec3'], 1, 0)

    E = params['embed']
    z = jnp.transpose(z_e, (0, 2, 3, 1))
    d2 = (jnp.sum(z * z, -1, keepdims=True)
          - 2.0 * jnp.einsum('bhwd,kd->bhwk', z, E)
          + jnp.sum(E * E, -1))
    z_index = jnp.argmin(d2, axis=-1)
    z_discrete = jax.nn.one_hot(z_index, K, dtype=jnp.int32)
    z_q = jnp.transpose(E[z_index], (0, 3, 1, 2))
    z_ss = z_q  # forward value of straight-through estimator

    h = _gelu(_block(z_ss, params['bd1']))
    h = _gelu(_block(h, params['bd2']))
    h = _gelu(_deconv(h, params['dt1'], 2, 2))
    x_pred = jax.nn.sigmoid(_deconv(h, params['dt2'], 0, 1))

    sse_x = jnp.sum((x - x_pred) ** 2)
    sse_z = jnp.sum((z_e - z_q) ** 2)
    return x_pred, z_discrete, z_index.astype(jnp.int32), sse_x, sse_z


_PMAP = None


def _get_pmap():
    global _PMAP
    if _PMAP is None:
        _PMAP = jax.pmap(_shard_forward, axis_name='dp',
                         devices=jax.devices()[:NCORES])
    return _PMAP


def kernel(x, params):
    x = np.asarray(x)
    params = jax.tree_util.tree_map(np.asarray, params)

    # shard batch across cores, replicate params
    xs = x.reshape(NCORES, B // NCORES, CH, H, W)
    params_rep = jax.tree_util.tree_map(
        lambda a: np.broadcast_to(a, (NCORES,) + a.shape), params)

    x_pred_s, z_disc_s, z_idx_s, sse_x_s, sse_z_s = _get_pmap()(xs, params_rep)

    x_pred = np.asarray(x_pred_s).reshape(B, CH, H, W)
    z_discrete = np.asarray(z_disc_s).reshape(B, H // 4, W // 4, K)
    z_index = np.asarray(z_idx_s).reshape(B, H // 4, W // 4)

    # finish loss reduction on host (the all-reduce of the data-parallel sum)
    sse_x = float(np.asarray(sse_x_s, np.float64).sum())
    sse_z = float(np.asarray(sse_z_s, np.float64).sum())
    n_x = B * CH * H * W
    n_z = B * D * (H // 4) * (W // 4)
    loss = (sse_x / n_x + 1.25 * (sse_z / n_z)) / B
    loss = np.float32(loss)

    return x_pred, z_discrete, loss, z_index
